# revision 20
# baseline (speedup 1.0000x reference)
"""Trainium2 Bass kernel: CustomFlashAttention (B=1, S=2048, D=2048, H=16, Hd=128).

Sharding (Megatron tensor-parallel over heads, 8 NeuronCores):
  - each core owns 2 heads (256 feature dims)
  - w_q/w_k/w_v column-parallel (pre-transposed + sliced on host)
  - w_o row-parallel; cores emit fp16 partial outputs, host sums the 8 partials

Device layout: activations feature-major ([feat, seq]) so every matmul's
contraction lands on SBUF partitions with no on-device transposes:
  qT/kT = W^T-projections of xT  [hd, s];  v natural [s, hd]
  scores transposed sT[k, q] = K Q^T; exp'd fp16 tiles feed P^T into PV.
  Softmax denominators: DVE accumulates the 16 exp'd tiles elementwise into
  pt_sum, then ONE ones-matmul per (chunk, head) broadcasts the cross-partition
  sum (vs one per tile) - saves ~26us of PE time.

Schedule: 24 warm-up matmuls un-throttle the PE clock while DMAs land; phase 1
computes k/v (all chunks) + q (chunk 0) consuming x d-tiles as they stream in;
phase 2 runs attention per (chunk, head) with the next chunk's q projection and
the previous chunk's output projection interleaved as filler matmuls between
the score and exp-dependent PV matmuls, so the in-order PE never waits on the
scalar engine's exp.

DMA: inputs as >=256KB transfers with >=2KB partition lines (host pre-packs
DRAM layouts to match SBUF tiles); output partials written fp16 (not fp32) in
512KB pieces with 4KB lines. All matmul operands fp16, accumulation fp32 PSUM.
"""

import sys
from contextlib import ExitStack

import numpy as np

if "/opt/trn_rl_repo" not in sys.path:
    sys.path.insert(0, "/opt/trn_rl_repo")

import concourse.bass as bass  # noqa: F401
import concourse.tile as tile
from concourse import bacc, mybir
from concourse.bass_utils import run_bass_kernel_spmd

P = 128                      # SBUF partitions
S = 2048                     # sequence length
D = 2048                     # hidden dim
H = 16                       # heads
HD = 128                     # head dim
NCORES = 8
HPC = H // NCORES            # heads per core = 2
HDC = HPC * HD               # feature dims per core = 256
DT = D // P                  # 16 contraction tiles
NCH = 4                      # seq chunks
CH = S // NCH                # 512
KT = S // P                  # 16 key tiles
SCALE = 1.0 / float(np.sqrt(HD))

f32 = mybir.dt.float32
f16 = mybir.dt.float16

_CACHE = {}
LAST_RESULT = None


def _build_nc():
    nc = bacc.Bacc("TRN2", target_bir_lowering=False, debug=False, num_devices=NCORES)

    # DRAM layouts pre-packed on host to give big contiguous partition lines.
    xin = nc.dram_tensor("xin", [DT, P, S], f16, kind="ExternalInput").ap()
    wkin = nc.dram_tensor("wkin", [P, DT, HDC], f16, kind="ExternalInput").ap()
    wqin = nc.dram_tensor("wqin", [P, DT, HDC], f16, kind="ExternalInput").ap()
    wvin = nc.dram_tensor("wvin", [P, DT, HDC], f16, kind="ExternalInput").ap()
    woin = nc.dram_tensor("woin", [P, HDC // P, D], f16, kind="ExternalInput").ap()
    outb = nc.dram_tensor("outb", [NCH, P, DT, CH], f16, kind="ExternalOutput").ap()

    with ExitStack() as ctx:
        tc = ctx.enter_context(tile.TileContext(nc))

        singles = ctx.enter_context(tc.tile_pool(name="singles", bufs=1))
        ptpool = ctx.enter_context(tc.tile_pool(name="pt", bufs=3))
        pspool = ctx.enter_context(tc.tile_pool(name="psum_sb", bufs=2))
        rspool = ctx.enter_context(tc.tile_pool(name="rs", bufs=2))
        obpool = ctx.enter_context(tc.tile_pool(name="ob", bufs=2))

        # Persistent SBUF tensors
        x_sb = singles.tile([P, DT, S], f16, tag="x")
        wq_sb = singles.tile([P, DT, HDC], f16, tag="wq")
        wk_sb = singles.tile([P, DT, HDC], f16, tag="wk")
        wv_sb = singles.tile([P, DT, HDC], f16, tag="wv")
        wo_sb = singles.tile([P, HDC // P, D], f16, tag="wo")
        qT_sb = singles.tile([P, HPC, S], f16, tag="qT")
        kT_sb = singles.tile([P, HPC, S], f16, tag="kT")
        v_sb = singles.tile([P, KT, HDC], f16, tag="v")
        oT_sb = singles.tile([P, HPC, S], f16, tag="oT")
        ones = singles.tile([P, P], f16, tag="ones")
        # operand for the warm-up matmuls (results discarded); memset on the
        # otherwise-idle GpSimd engine so the PE can start the moment its
        # sequencer reaches main, without waiting on the DVE
        wjunk = singles.tile([P, CH], f16, tag="wjunk")

        nc.gpsimd.memset(wjunk, 0.0)
        nc.vector.memset(ones, 1.0)

        # ---- input DMAs: one need-ordered stream split over the two HWDGE
        # rings (SP + ACT), which complete transfers concurrently. 512KB
        # full d-rows amortize the per-transfer completion receipt. wq/wo land
        # last (q(c0) runs late in phase 1; wo in phase 2). gpsimd (SWDGE) is
        # reserved for output writes so it never steals early packet slots.
        def xd(d):
            return lambda q: q.dma_start(out=x_sb[:, d, :], in_=xin[d])

        def wpiece(sb, dram, lo, hi):
            return lambda q: q.dma_start(out=sb[:, lo:hi, :], in_=dram[:, lo:hi, :])

        stream = []
        for q4 in range(4):
            stream += [wpiece(wk_sb, wkin, q4 * 4, q4 * 4 + 4),
                       xd(q4 * 4),
                       wpiece(wq_sb, wqin, q4 * 4, q4 * 4 + 4),
                       xd(q4 * 4 + 1),
                       wpiece(wv_sb, wvin, q4 * 4, q4 * 4 + 4),
                       xd(q4 * 4 + 2), xd(q4 * 4 + 3)]
        stream.append(lambda q: q.dma_start(out=wo_sb, in_=woin))
        for i, item in enumerate(stream):
            item(nc.sync if i % 2 == 0 else nc.scalar)

        # ---------- Phase 1: k/v all chunks, q chunk 0 ----------
        p1_ctx = ExitStack()
        k_ps = p1_ctx.enter_context(tc.tile_pool(name="kps", bufs=2, space="PSUM"))
        q_ps = p1_ctx.enter_context(tc.tile_pool(name="qps", bufs=2, space="PSUM"))
        v_ps = p1_ctx.enter_context(tc.tile_pool(name="vps", bufs=4, space="PSUM"))

        # warm-up matmuls: un-throttle the PE HAM clock gate while DMAs land.
        # Operands are memset junk with no DMA dependencies, so the PE starts
        # the moment its sequencer reaches main.
        warm = k_ps.tile([P, CH], f32, tag="pk", name="warm")
        for i in range(8):
            nc.tensor.matmul(warm, lhsT=wjunk[:, 0:P], rhs=wjunk,
                             start=(i == 0), stop=(i == 7))

        for c in range(NCH):
            csl = slice(c * CH, (c + 1) * CH)
            pk = [k_ps.tile([P, CH], f32, tag="pk", name=f"pk{c}_{h}")
                  for h in range(HPC)]
            pv = [v_ps.tile([P, HDC], f32, tag="pv", name=f"pv{c}_{st}")
                  for st in range(4)]
            pq = ([q_ps.tile([P, CH], f32, tag="pq", name=f"pq{c}_{h}")
                   for h in range(HPC)] if c == 0 else None)
            for d in range(DT):
                first, last = (d == 0), (d == DT - 1)
                for h in range(HPC):
                    nc.tensor.matmul(
                        pk[h], lhsT=wk_sb[:, d, h * HD:(h + 1) * HD],
                        rhs=x_sb[:, d, csl], start=first, stop=last)
                if last:
                    for h in range(HPC):
                        nc.vector.tensor_copy(kT_sb[:, h, csl], pk[h])
                if pq is not None:
                    for h in range(HPC):
                        nc.tensor.matmul(
                            pq[h], lhsT=wq_sb[:, d, h * HD:(h + 1) * HD],
                            rhs=x_sb[:, d, csl], start=first, stop=last)
                    if last:
                        for h in range(HPC):
                            nc.vector.tensor_copy(qT_sb[:, h, csl], pq[h])
                for st in range(4):
                    nc.tensor.matmul(
                        pv[st],
                        lhsT=x_sb[:, d, c * CH + st * P:c * CH + (st + 1) * P],
                        rhs=wv_sb[:, d, :], start=first, stop=last)
                if last:
                    for st in range(4):
                        # scalar engine is idle in phase 1: drain v there so
                        # chunk boundaries don't serialize on the DVE
                        nc.scalar.copy(v_sb[:, c * 4 + st, :], pv[st])

        p1_ctx.close()  # release phase-1 PSUM banks

        # ---------- Phase 2 PSUM pools ----------
        # deferred-q accum and the denominator tile are serially live within a
        # unit, so they share one rotating bank (tag "scr"); that frees a bank
        # to double-buffer the wo projection PSUM.
        wo_ps = ctx.enter_context(tc.tile_pool(name="wops", bufs=2, space="PSUM"))
        p2_ctx = ExitStack()
        sc_ps = p2_ctx.enter_context(tc.tile_pool(name="scps", bufs=2, space="PSUM"))
        o_ps = p2_ctx.enter_context(tc.tile_pool(name="ops", bufs=1, space="PSUM"))
        scr_ps = p2_ctx.enter_context(tc.tile_pool(name="scrps", bufs=1, space="PSUM"))

        def make_defq_units(c, h):
            """16 single-matmul units projecting q for (c, h); last drains."""
            csl = slice(c * CH, (c + 1) * CH)
            pqd = scr_ps.tile([P, CH], f32, tag="scr", name=f"dpq{c}_{h}")

            def unit(d):
                def emit():
                    nc.tensor.matmul(
                        pqd, lhsT=wq_sb[:, d, h * HD:(h + 1) * HD],
                        rhs=x_sb[:, d, csl],
                        start=(d == 0), stop=(d == DT - 1))
                    if d == DT - 1:
                        nc.vector.tensor_copy(qT_sb[:, h, csl], pqd)
                return emit

            return [unit(d) for d in range(DT)]

        def make_wo_units(c, ob, pool):
            """32 single-matmul units for chunk c's output projection.
            ob: [P, DT, CH] f16 SBUF chunk buffer; DMA per 4-ot quad."""
            csl = slice(c * CH, (c + 1) * CH)
            state = {}

            def unit(ot, di):
                def emit():
                    if di == 0:
                        state[ot] = pool.tile([P, CH], f32, tag="pout",
                                              name=f"po{c}_{ot}")
                    pout = state[ot]
                    nc.tensor.matmul(
                        pout, lhsT=wo_sb[:, di, ot * P:(ot + 1) * P],
                        rhs=oT_sb[:, di, csl],
                        start=(di == 0), stop=(di == HDC // P - 1))
                    if di == HDC // P - 1:
                        if ot % 2 == 0:
                            nc.vector.tensor_copy(ob[:, ot, :], pout)
                        else:
                            nc.scalar.copy(ob[:, ot, :], pout)
                        if ot % 4 == 3:
                            nc.gpsimd.dma_start(
                                out=outb[c][:, ot - 3:ot + 1, :],
                                in_=ob[:, ot - 3:ot + 1, :])
                return emit

            return [unit(ot, di) for ot in range(DT) for di in range(HDC // P)]

        # ---------- Phase 2: attention with interleaved fillers ----------
        def attention(c, h, fillers, fills_per_g):
            csl = slice(c * CH, (c + 1) * CH)
            po = o_ps.tile([P, CH], f32, tag="po", name=f"pa{c}_{h}")
            ptsum = pspool.tile([P, 2, CH], f16, tag="ptsum", name=f"pts{c}_{h}")
            fi = 0
            for g in range(KT // 2):
                psc = sc_ps.tile([P, 2, CH], f32, tag="psc", name=f"psc{c}_{h}_{g}")
                for j in range(2):
                    kj = g * 2 + j
                    nc.tensor.matmul(
                        psc[:, j, :], lhsT=kT_sb[:, h, kj * P:(kj + 1) * P],
                        rhs=qT_sb[:, h, csl], start=True, stop=True)
                # independent PE work here hides the exp latency
                for _ in range(fills_per_g[g]):
                    if fi < len(fillers):
                        fillers[fi]()
                        fi += 1
                pt = ptpool.tile([P, 2, CH], f16, tag="pt", name=f"pt{c}_{h}_{g}")
                nc.scalar.activation(
                    out=pt, in_=psc,
                    func=mybir.ActivationFunctionType.Exp, scale=SCALE)
                for j in range(2):
                    kj = g * 2 + j
                    nc.tensor.matmul(
                        po, lhsT=v_sb[:, kj, h * HD:(h + 1) * HD],
                        rhs=pt[:, j, :], start=(kj == 0), stop=(kj == KT - 1))
                if g == 0:
                    nc.vector.tensor_copy(ptsum, pt)
                else:
                    nc.vector.tensor_add(ptsum, ptsum, pt)
            while fi < len(fillers):
                fillers[fi]()
                fi += 1
            with tc.high_priority():
                ptf = pspool.tile([P, CH], f16, tag="ptf", name=f"ptf{c}_{h}")
                nc.vector.tensor_add(ptf, ptsum[:, 0, :], ptsum[:, 1, :])
                pden = scr_ps.tile([P, CH], f32, tag="scr", name=f"pd{c}_{h}")
                nc.tensor.matmul(pden, lhsT=ones, rhs=ptf, start=True, stop=True)
                rs = rspool.tile([P, CH], f32, tag="rs", name=f"rs{c}_{h}")
                nc.vector.reciprocal_approx_fast(out=rs, in_=pden)
                nc.vector.tensor_mul(oT_sb[:, h, csl], po, rs)

        obs = {}
        for c in range(NCH):
            if c > 0:
                obs[c - 1] = obpool.tile([P, DT, CH], f16, tag="ob",
                                         name=f"ob{c - 1}")
            for h in range(HPC):
                fills = []
                if c + 1 < NCH:
                    fills += make_defq_units(c + 1, h)        # 16 units
                if c > 0:
                    wo_units = make_wo_units(c - 1, obs[c - 1], wo_ps) \
                        if h == 0 else obs[(c - 1, "units")]
                    obs[(c - 1, "units")] = wo_units
                    half = len(wo_units) // 2                  # 16 units
                    fills += wo_units[:half] if h == 0 else wo_units[half:]
                n = len(fills)
                if (c, h) == (NCH - 1, HPC - 1):
                    # last unit: hold back fillers to cover the final
                    # denominator chain's latency before the tail starts
                    n = max(0, n - 3)
                base, extra = n // 8, n % 8
                pat = [base + (1 if g < extra else 0) for g in range(8)]
                attention(c, h, fills, pat)
        # tail: last chunk's output projection with deep PSUM buffering
        # (attention pools released first so four pout banks can rotate)
        p2_ctx.close()
        tail_ps = ctx.enter_context(tc.tile_pool(name="tailps", bufs=6, space="PSUM"))
        obs[NCH - 1] = obpool.tile([P, DT, CH], f16, tag="ob", name=f"ob{NCH - 1}")
        for u in make_wo_units(NCH - 1, obs[NCH - 1], tail_ps):
            u()

    nc.compile()
    return nc


def _get_nc():
    if "nc" not in _CACHE:
        _CACHE["nc"] = _build_nc()
    return _CACHE["nc"]


def make_in_maps(x, w_q, w_k, w_v, w_o):
    x = np.asarray(x, dtype=np.float32).reshape(S, D)
    w_q = np.asarray(w_q, dtype=np.float32)
    w_k = np.asarray(w_k, dtype=np.float32)
    w_v = np.asarray(w_v, dtype=np.float32)
    w_o = np.asarray(w_o, dtype=np.float32)
    xT = x.T.astype(np.float16)                      # [D, S]
    # xin[dt, p, s] = xT[dt*128 + p, s]
    xin = np.ascontiguousarray(xT.reshape(DT, P, S))

    def pack_w(w_slice):                             # w_slice: [HDC, D]
        wT = w_slice.T.astype(np.float16)            # [D, HDC]
        # win[p, dt, h] = wT[dt*128 + p, h]
        return np.ascontiguousarray(wT.reshape(DT, P, HDC).transpose(1, 0, 2))

    in_maps = []
    for r in range(NCORES):
        hs = slice(r * HDC, (r + 1) * HDC)
        woT = w_o[:, hs].T.astype(np.float16)        # [HDC, D]
        # woin[p, di, o] = woT[di*128 + p, o]
        woin = np.ascontiguousarray(
            woT.reshape(HDC // P, P, D).transpose(1, 0, 2))
        in_maps.append({
            "xin": xin,
            "wqin": pack_w(w_q[hs, :]),
            "wkin": pack_w(w_k[hs, :]),
            "wvin": pack_w(w_v[hs, :]),
            "woin": woin,
        })
    return in_maps


def kernel(x, w_q, w_k, w_v, w_o):
    global LAST_RESULT
    in_maps = make_in_maps(x, w_q, w_k, w_v, w_o)
    nc = _get_nc()
    res = run_bass_kernel_spmd(nc, in_maps, core_ids=list(range(NCORES)))
    LAST_RESULT = res
    acc = np.zeros((D, S), dtype=np.float32)
    for r in res.results:
        ob = r["outb"].astype(np.float32)            # [NCH, P, DT, CH]
        # outT[ot*128 + p, c*512 + j] = ob[c, p, ot, j]
        acc += ob.transpose(2, 1, 0, 3).reshape(D, S)
    return np.ascontiguousarray(acc.T).astype(np.float32).reshape(1, S, D)


# revision 21
# speedup vs baseline: 1.0300x; 1.0300x over previous
"""Trainium2 Bass kernel: CustomFlashAttention (B=1, S=2048, D=2048, H=16, Hd=128).

Sharding (Megatron tensor-parallel over heads, 8 NeuronCores):
  - each core owns 2 heads (256 feature dims)
  - w_q/w_k/w_v column-parallel (pre-transposed + sliced on host)
  - w_o row-parallel; cores emit fp16 partial outputs, host sums the 8 partials

Device layout: activations feature-major ([feat, seq]) so every matmul's
contraction lands on SBUF partitions with no on-device transposes:
  qT/kT = W^T-projections of xT  [hd, s];  v natural [s, hd]
  scores transposed sT[k, q] = K Q^T; exp'd fp16 tiles feed P^T into PV.
  Softmax denominators: DVE accumulates the 16 exp'd tiles elementwise into
  pt_sum, then ONE ones-matmul per (chunk, head) broadcasts the cross-partition
  sum (vs one per tile) - saves ~26us of PE time.

Schedule: 24 warm-up matmuls un-throttle the PE clock while DMAs land; phase 1
computes k/v (all chunks) + q (chunk 0) consuming x d-tiles as they stream in;
phase 2 runs attention per (chunk, head) with the next chunk's q projection and
the previous chunk's output projection interleaved as filler matmuls between
the score and exp-dependent PV matmuls, so the in-order PE never waits on the
scalar engine's exp.

DMA: inputs as >=256KB transfers with >=2KB partition lines (host pre-packs
DRAM layouts to match SBUF tiles); output partials written fp16 (not fp32) in
512KB pieces with 4KB lines. All matmul operands fp16, accumulation fp32 PSUM.
"""

import sys
from contextlib import ExitStack

import numpy as np

if "/opt/trn_rl_repo" not in sys.path:
    sys.path.insert(0, "/opt/trn_rl_repo")

import concourse.bass as bass  # noqa: F401
import concourse.tile as tile
from concourse import bacc, mybir
from concourse.bass_utils import run_bass_kernel_spmd

P = 128                      # SBUF partitions
S = 2048                     # sequence length
D = 2048                     # hidden dim
H = 16                       # heads
HD = 128                     # head dim
NCORES = 8
HPC = H // NCORES            # heads per core = 2
HDC = HPC * HD               # feature dims per core = 256
DT = D // P                  # 16 contraction tiles
NCH = 4                      # seq chunks
CH = S // NCH                # 512
KT = S // P                  # 16 key tiles
SCALE = 1.0 / float(np.sqrt(HD))

f32 = mybir.dt.float32
f16 = mybir.dt.float16

_CACHE = {}
LAST_RESULT = None


def _build_nc():
    nc = bacc.Bacc("TRN2", target_bir_lowering=False, debug=False, num_devices=NCORES)

    # DRAM layouts pre-packed on host to give big contiguous partition lines.
    xin = nc.dram_tensor("xin", [DT, P, S], f16, kind="ExternalInput").ap()
    wkin = nc.dram_tensor("wkin", [P, DT, HDC], f16, kind="ExternalInput").ap()
    wqin = nc.dram_tensor("wqin", [P, DT, HDC], f16, kind="ExternalInput").ap()
    wvin = nc.dram_tensor("wvin", [P, DT, HDC], f16, kind="ExternalInput").ap()
    woin = nc.dram_tensor("woin", [P, HDC // P, D], f16, kind="ExternalInput").ap()
    outb = nc.dram_tensor("outb", [NCH, P, DT, CH], f16, kind="ExternalOutput").ap()

    with ExitStack() as ctx:
        tc = ctx.enter_context(tile.TileContext(nc))

        singles = ctx.enter_context(tc.tile_pool(name="singles", bufs=1))
        ptpool = ctx.enter_context(tc.tile_pool(name="pt", bufs=3))
        pspool = ctx.enter_context(tc.tile_pool(name="psum_sb", bufs=2))
        rspool = ctx.enter_context(tc.tile_pool(name="rs", bufs=2))
        obpool = ctx.enter_context(tc.tile_pool(name="ob", bufs=2))

        # Persistent SBUF tensors
        x_sb = singles.tile([P, DT, S], f16, tag="x")
        wq_sb = singles.tile([P, DT, HDC], f16, tag="wq")
        wk_sb = singles.tile([P, DT, HDC], f16, tag="wk")
        wv_sb = singles.tile([P, DT, HDC], f16, tag="wv")
        wo_sb = singles.tile([P, HDC // P, D], f16, tag="wo")
        qT_sb = singles.tile([P, HPC, S], f16, tag="qT")
        kT_sb = singles.tile([P, HPC, S], f16, tag="kT")
        v_sb = singles.tile([P, KT, HDC], f16, tag="v")
        oT_sb = singles.tile([P, HPC, S], f16, tag="oT")
        ones = singles.tile([P, P], f16, tag="ones")
        # operand for the warm-up matmuls (results discarded); memset on the
        # otherwise-idle GpSimd engine so the PE can start the moment its
        # sequencer reaches main, without waiting on the DVE
        wjunk = singles.tile([P, CH], f16, tag="wjunk")

        nc.gpsimd.memset(wjunk, 0.0)
        nc.vector.memset(ones, 1.0)

        # ---- input DMAs: one need-ordered stream split over the two HWDGE
        # rings (SP + ACT), which complete transfers concurrently. 512KB
        # full d-rows amortize the per-transfer completion receipt. wq/wo land
        # last (q(c0) runs late in phase 1; wo in phase 2). gpsimd (SWDGE) is
        # reserved for output writes so it never steals early packet slots.
        def xd(d):
            return lambda q: q.dma_start(out=x_sb[:, d, :], in_=xin[d])

        def wpiece(sb, dram, lo, hi):
            return lambda q: q.dma_start(out=sb[:, lo:hi, :], in_=dram[:, lo:hi, :])

        stream = []
        for q4 in range(4):
            stream += [wpiece(wk_sb, wkin, q4 * 4, q4 * 4 + 4),
                       xd(q4 * 4),
                       wpiece(wq_sb, wqin, q4 * 4, q4 * 4 + 4),
                       xd(q4 * 4 + 1),
                       wpiece(wv_sb, wvin, q4 * 4, q4 * 4 + 4),
                       xd(q4 * 4 + 2), xd(q4 * 4 + 3)]
        stream.append(lambda q: q.dma_start(out=wo_sb, in_=woin))
        for item in stream:
            item(nc.sync)

        # ---------- Phase 1: k/v all chunks, q chunk 0 ----------
        p1_ctx = ExitStack()
        k_ps = p1_ctx.enter_context(tc.tile_pool(name="kps", bufs=2, space="PSUM"))
        q_ps = p1_ctx.enter_context(tc.tile_pool(name="qps", bufs=2, space="PSUM"))
        v_ps = p1_ctx.enter_context(tc.tile_pool(name="vps", bufs=4, space="PSUM"))

        # warm-up matmuls: un-throttle the PE HAM clock gate while DMAs land.
        # Operands are memset junk with no DMA dependencies, so the PE starts
        # the moment its sequencer reaches main.
        warm = k_ps.tile([P, CH], f32, tag="pk", name="warm")
        for i in range(8):
            nc.tensor.matmul(warm, lhsT=wjunk[:, 0:P], rhs=wjunk,
                             start=(i == 0), stop=(i == 7))

        for c in range(NCH):
            csl = slice(c * CH, (c + 1) * CH)
            pk = [k_ps.tile([P, CH], f32, tag="pk", name=f"pk{c}_{h}")
                  for h in range(HPC)]
            pv = [v_ps.tile([P, HDC], f32, tag="pv", name=f"pv{c}_{st}")
                  for st in range(4)]
            pq = ([q_ps.tile([P, CH], f32, tag="pq", name=f"pq{c}_{h}")
                   for h in range(HPC)] if c == 0 else None)
            for d in range(DT):
                first, last = (d == 0), (d == DT - 1)
                for h in range(HPC):
                    nc.tensor.matmul(
                        pk[h], lhsT=wk_sb[:, d, h * HD:(h + 1) * HD],
                        rhs=x_sb[:, d, csl], start=first, stop=last)
                if last:
                    for h in range(HPC):
                        nc.vector.tensor_copy(kT_sb[:, h, csl], pk[h])
                if pq is not None:
                    for h in range(HPC):
                        nc.tensor.matmul(
                            pq[h], lhsT=wq_sb[:, d, h * HD:(h + 1) * HD],
                            rhs=x_sb[:, d, csl], start=first, stop=last)
                    if last:
                        for h in range(HPC):
                            nc.vector.tensor_copy(qT_sb[:, h, csl], pq[h])
                for st in range(4):
                    nc.tensor.matmul(
                        pv[st],
                        lhsT=x_sb[:, d, c * CH + st * P:c * CH + (st + 1) * P],
                        rhs=wv_sb[:, d, :], start=first, stop=last)
                if last:
                    for st in range(4):
                        # scalar engine is idle in phase 1: drain v there so
                        # chunk boundaries don't serialize on the DVE
                        nc.scalar.copy(v_sb[:, c * 4 + st, :], pv[st])

        p1_ctx.close()  # release phase-1 PSUM banks

        # ---------- Phase 2 PSUM pools ----------
        # deferred-q accum and the denominator tile are serially live within a
        # unit, so they share one rotating bank (tag "scr"); that frees a bank
        # to double-buffer the wo projection PSUM.
        wo_ps = ctx.enter_context(tc.tile_pool(name="wops", bufs=2, space="PSUM"))
        p2_ctx = ExitStack()
        sc_ps = p2_ctx.enter_context(tc.tile_pool(name="scps", bufs=2, space="PSUM"))
        o_ps = p2_ctx.enter_context(tc.tile_pool(name="ops", bufs=1, space="PSUM"))
        scr_ps = p2_ctx.enter_context(tc.tile_pool(name="scrps", bufs=1, space="PSUM"))

        def make_defq_units(c, h):
            """16 single-matmul units projecting q for (c, h); last drains."""
            csl = slice(c * CH, (c + 1) * CH)
            pqd = scr_ps.tile([P, CH], f32, tag="scr", name=f"dpq{c}_{h}")

            def unit(d):
                def emit():
                    nc.tensor.matmul(
                        pqd, lhsT=wq_sb[:, d, h * HD:(h + 1) * HD],
                        rhs=x_sb[:, d, csl],
                        start=(d == 0), stop=(d == DT - 1))
                    if d == DT - 1:
                        nc.vector.tensor_copy(qT_sb[:, h, csl], pqd)
                return emit

            return [unit(d) for d in range(DT)]

        def make_wo_units(c, ob, pool):
            """32 single-matmul units for chunk c's output projection.
            ob: [P, DT, CH] f16 SBUF chunk buffer; DMA per 4-ot quad."""
            csl = slice(c * CH, (c + 1) * CH)
            state = {}

            def unit(ot, di):
                def emit():
                    if di == 0:
                        state[ot] = pool.tile([P, CH], f32, tag="pout",
                                              name=f"po{c}_{ot}")
                    pout = state[ot]
                    nc.tensor.matmul(
                        pout, lhsT=wo_sb[:, di, ot * P:(ot + 1) * P],
                        rhs=oT_sb[:, di, csl],
                        start=(di == 0), stop=(di == HDC // P - 1))
                    if di == HDC // P - 1:
                        if ot % 2 == 0:
                            nc.vector.tensor_copy(ob[:, ot, :], pout)
                        else:
                            nc.scalar.copy(ob[:, ot, :], pout)
                        if ot % 4 == 3:
                            nc.gpsimd.dma_start(
                                out=outb[c][:, ot - 3:ot + 1, :],
                                in_=ob[:, ot - 3:ot + 1, :])
                return emit

            return [unit(ot, di) for ot in range(DT) for di in range(HDC // P)]

        # ---------- Phase 2: attention with interleaved fillers ----------
        def attention(c, h, fillers, fills_per_g):
            csl = slice(c * CH, (c + 1) * CH)
            po = o_ps.tile([P, CH], f32, tag="po", name=f"pa{c}_{h}")
            ptsum = pspool.tile([P, 2, CH], f16, tag="ptsum", name=f"pts{c}_{h}")
            fi = 0
            for g in range(KT // 2):
                psc = sc_ps.tile([P, 2, CH], f32, tag="psc", name=f"psc{c}_{h}_{g}")
                for j in range(2):
                    kj = g * 2 + j
                    nc.tensor.matmul(
                        psc[:, j, :], lhsT=kT_sb[:, h, kj * P:(kj + 1) * P],
                        rhs=qT_sb[:, h, csl], start=True, stop=True)
                # independent PE work here hides the exp latency
                for _ in range(fills_per_g[g]):
                    if fi < len(fillers):
                        fillers[fi]()
                        fi += 1
                pt = ptpool.tile([P, 2, CH], f16, tag="pt", name=f"pt{c}_{h}_{g}")
                nc.scalar.activation(
                    out=pt, in_=psc,
                    func=mybir.ActivationFunctionType.Exp, scale=SCALE)
                for j in range(2):
                    kj = g * 2 + j
                    nc.tensor.matmul(
                        po, lhsT=v_sb[:, kj, h * HD:(h + 1) * HD],
                        rhs=pt[:, j, :], start=(kj == 0), stop=(kj == KT - 1))
                if g == 0:
                    nc.vector.tensor_copy(ptsum, pt)
                else:
                    nc.vector.tensor_add(ptsum, ptsum, pt)
            while fi < len(fillers):
                fillers[fi]()
                fi += 1
            with tc.high_priority():
                ptf = pspool.tile([P, CH], f16, tag="ptf", name=f"ptf{c}_{h}")
                nc.vector.tensor_add(ptf, ptsum[:, 0, :], ptsum[:, 1, :])
                pden = scr_ps.tile([P, CH], f32, tag="scr", name=f"pd{c}_{h}")
                nc.tensor.matmul(pden, lhsT=ones, rhs=ptf, start=True, stop=True)
                rs = rspool.tile([P, CH], f32, tag="rs", name=f"rs{c}_{h}")
                nc.vector.reciprocal_approx_fast(out=rs, in_=pden)
                nc.vector.tensor_mul(oT_sb[:, h, csl], po, rs)

        obs = {}
        for c in range(NCH):
            if c > 0:
                obs[c - 1] = obpool.tile([P, DT, CH], f16, tag="ob",
                                         name=f"ob{c - 1}")
            for h in range(HPC):
                fills = []
                if c + 1 < NCH:
                    fills += make_defq_units(c + 1, h)        # 16 units
                if c > 0:
                    wo_units = make_wo_units(c - 1, obs[c - 1], wo_ps) \
                        if h == 0 else obs[(c - 1, "units")]
                    obs[(c - 1, "units")] = wo_units
                    half = len(wo_units) // 2                  # 16 units
                    fills += wo_units[:half] if h == 0 else wo_units[half:]
                n = len(fills)
                if (c, h) == (NCH - 1, HPC - 1):
                    # last unit: hold back fillers to cover the final
                    # denominator chain's latency before the tail starts
                    n = max(0, n - 3)
                base, extra = n // 8, n % 8
                pat = [base + (1 if g < extra else 0) for g in range(8)]
                attention(c, h, fills, pat)
        # tail: last chunk's output projection with deep PSUM buffering
        # (attention pools released first so four pout banks can rotate)
        p2_ctx.close()
        tail_ps = ctx.enter_context(tc.tile_pool(name="tailps", bufs=6, space="PSUM"))
        obs[NCH - 1] = obpool.tile([P, DT, CH], f16, tag="ob", name=f"ob{NCH - 1}")
        for u in make_wo_units(NCH - 1, obs[NCH - 1], tail_ps):
            u()

    nc.compile()
    return nc


def _get_nc():
    if "nc" not in _CACHE:
        _CACHE["nc"] = _build_nc()
    return _CACHE["nc"]


def make_in_maps(x, w_q, w_k, w_v, w_o):
    x = np.asarray(x, dtype=np.float32).reshape(S, D)
    w_q = np.asarray(w_q, dtype=np.float32)
    w_k = np.asarray(w_k, dtype=np.float32)
    w_v = np.asarray(w_v, dtype=np.float32)
    w_o = np.asarray(w_o, dtype=np.float32)
    xT = x.T.astype(np.float16)                      # [D, S]
    # xin[dt, p, s] = xT[dt*128 + p, s]
    xin = np.ascontiguousarray(xT.reshape(DT, P, S))

    def pack_w(w_slice):                             # w_slice: [HDC, D]
        wT = w_slice.T.astype(np.float16)            # [D, HDC]
        # win[p, dt, h] = wT[dt*128 + p, h]
        return np.ascontiguousarray(wT.reshape(DT, P, HDC).transpose(1, 0, 2))

    in_maps = []
    for r in range(NCORES):
        hs = slice(r * HDC, (r + 1) * HDC)
        woT = w_o[:, hs].T.astype(np.float16)        # [HDC, D]
        # woin[p, di, o] = woT[di*128 + p, o]
        woin = np.ascontiguousarray(
            woT.reshape(HDC // P, P, D).transpose(1, 0, 2))
        in_maps.append({
            "xin": xin,
            "wqin": pack_w(w_q[hs, :]),
            "wkin": pack_w(w_k[hs, :]),
            "wvin": pack_w(w_v[hs, :]),
            "woin": woin,
        })
    return in_maps


def kernel(x, w_q, w_k, w_v, w_o):
    global LAST_RESULT
    in_maps = make_in_maps(x, w_q, w_k, w_v, w_o)
    nc = _get_nc()
    res = run_bass_kernel_spmd(nc, in_maps, core_ids=list(range(NCORES)))
    LAST_RESULT = res
    acc = np.zeros((D, S), dtype=np.float32)
    for r in res.results:
        ob = r["outb"].astype(np.float32)            # [NCH, P, DT, CH]
        # outT[ot*128 + p, c*512 + j] = ob[c, p, ot, j]
        acc += ob.transpose(2, 1, 0, 3).reshape(D, S)
    return np.ascontiguousarray(acc.T).astype(np.float32).reshape(1, S, D)


# revision 24
# speedup vs baseline: 1.0714x; 1.0402x over previous
"""Trainium2 Bass kernel: CustomFlashAttention (B=1, S=2048, D=2048, H=16, Hd=128).

Sharding (Megatron tensor-parallel over heads, 8 NeuronCores):
  - each core owns 2 heads (256 feature dims)
  - w_q/w_k/w_v column-parallel (pre-transposed + sliced on host)
  - w_o row-parallel; cores emit fp16 partial outputs, host sums the 8 partials

Device layout: activations feature-major ([feat, seq]) so every matmul's
contraction lands on SBUF partitions with no on-device transposes:
  qT/kT = W^T-projections of xT  [hd, s];  v natural [s, hd]
  scores transposed sT[k, q] = K Q^T; exp'd fp16 tiles feed P^T into PV.
  Softmax denominators: DVE accumulates the 16 exp'd tiles elementwise into
  pt_sum, then ONE ones-matmul per (chunk, head) broadcasts the cross-partition
  sum (vs one per tile) - saves ~26us of PE time.

Schedule: 24 warm-up matmuls un-throttle the PE clock while DMAs land; phase 1
computes k/v (all chunks) + q (chunk 0) consuming x d-tiles as they stream in;
phase 2 runs attention per (chunk, head) with the next chunk's q projection and
the previous chunk's output projection interleaved as filler matmuls between
the score and exp-dependent PV matmuls, so the in-order PE never waits on the
scalar engine's exp.

DMA: inputs as >=256KB transfers with >=2KB partition lines (host pre-packs
DRAM layouts to match SBUF tiles); output partials written fp16 (not fp32) in
512KB pieces with 4KB lines. All matmul operands fp16, accumulation fp32 PSUM.
"""

import sys
from contextlib import ExitStack

import numpy as np

if "/opt/trn_rl_repo" not in sys.path:
    sys.path.insert(0, "/opt/trn_rl_repo")

import concourse.bass as bass  # noqa: F401
import concourse.tile as tile
from concourse import bacc, mybir
from concourse.bass_utils import run_bass_kernel_spmd

P = 128                      # SBUF partitions
S = 2048                     # sequence length
D = 2048                     # hidden dim
H = 16                       # heads
HD = 128                     # head dim
NCORES = 8
HPC = H // NCORES            # heads per core = 2
HDC = HPC * HD               # feature dims per core = 256
DT = D // P                  # 16 contraction tiles
NCH = 4                      # seq chunks
CH = S // NCH                # 512
KT = S // P                  # 16 key tiles
SCALE = 1.0 / float(np.sqrt(HD))

f32 = mybir.dt.float32
f16 = mybir.dt.float16

_CACHE = {}
LAST_RESULT = None


def _build_nc():
    nc = bacc.Bacc("TRN2", target_bir_lowering=False, debug=False, num_devices=NCORES)

    # DRAM layouts pre-packed on host to give big contiguous partition lines.
    xin = nc.dram_tensor("xin", [2, DT, P, S // 2], f16, kind="ExternalInput").ap()
    wkin = nc.dram_tensor("wkin", [P, DT, HDC], f16, kind="ExternalInput").ap()
    wqin = nc.dram_tensor("wqin", [P, DT, HDC], f16, kind="ExternalInput").ap()
    wvin = nc.dram_tensor("wvin", [P, DT, HDC], f16, kind="ExternalInput").ap()
    woin = nc.dram_tensor("woin", [P, HDC // P, D], f16, kind="ExternalInput").ap()
    outb = nc.dram_tensor("outb", [NCH, P, DT, CH], f16, kind="ExternalOutput").ap()

    with ExitStack() as ctx:
        tc = ctx.enter_context(tile.TileContext(nc))

        singles = ctx.enter_context(tc.tile_pool(name="singles", bufs=1))
        ptpool = ctx.enter_context(tc.tile_pool(name="pt", bufs=3))
        pspool = ctx.enter_context(tc.tile_pool(name="psum_sb", bufs=2))
        rspool = ctx.enter_context(tc.tile_pool(name="rs", bufs=2))
        obpool = ctx.enter_context(tc.tile_pool(name="ob", bufs=2))

        # Persistent SBUF tensors
        x_sb = singles.tile([P, DT, S], f16, tag="x")
        wq_sb = singles.tile([P, DT, HDC], f16, tag="wq")
        wk_sb = singles.tile([P, DT, HDC], f16, tag="wk")
        wv_sb = singles.tile([P, DT, HDC], f16, tag="wv")
        wo_sb = singles.tile([P, HDC // P, D], f16, tag="wo")
        qT_sb = singles.tile([P, HPC, S], f16, tag="qT")
        kT_sb = singles.tile([P, HPC, S], f16, tag="kT")
        v_sb = singles.tile([P, KT, HDC], f16, tag="v")
        oT_sb = singles.tile([P, HPC, S], f16, tag="oT")
        ones = singles.tile([P, P], f16, tag="ones")
        # operand for the warm-up matmuls (results discarded); memset on the
        # otherwise-idle GpSimd engine so the PE can start the moment its
        # sequencer reaches main, without waiting on the DVE
        wjunk = singles.tile([P, CH], f16, tag="wjunk")

        nc.gpsimd.memset(wjunk, 0.0)
        nc.vector.memset(ones, 1.0)

        # ---- input DMAs: one need-ordered stream on the sync HWDGE ring.
        # 256KB pieces (2KB partition lines); chunk-0/1-critical data first
        # (weights quarter-by-quarter interleaved with x half-rows), then the
        # chunk-2/3 x halves, then wo. gpsimd (SWDGE) is reserved for output
        # writes so it never steals early packet slots.
        for q4 in range(4):
            dsl = slice(q4 * 4, (q4 + 1) * 4)
            nc.sync.dma_start(out=wk_sb[:, dsl, :], in_=wkin[:, dsl, :])
            nc.sync.dma_start(out=x_sb[:, q4 * 4, 0:S // 2], in_=xin[0][q4 * 4])
            nc.sync.dma_start(out=wq_sb[:, dsl, :], in_=wqin[:, dsl, :])
            nc.sync.dma_start(out=x_sb[:, q4 * 4 + 1, 0:S // 2],
                              in_=xin[0][q4 * 4 + 1])
            nc.sync.dma_start(out=wv_sb[:, dsl, :], in_=wvin[:, dsl, :])
            for d in range(q4 * 4 + 2, q4 * 4 + 4):
                nc.sync.dma_start(out=x_sb[:, d, 0:S // 2], in_=xin[0][d])
        for d in range(DT):
            nc.sync.dma_start(out=x_sb[:, d, S // 2:S], in_=xin[1][d])
        nc.sync.dma_start(out=wo_sb, in_=woin)

        # ---------- Phase 1: k/v all chunks, q chunk 0 ----------
        p1_ctx = ExitStack()
        k_ps = p1_ctx.enter_context(tc.tile_pool(name="kps", bufs=2, space="PSUM"))
        q_ps = p1_ctx.enter_context(tc.tile_pool(name="qps", bufs=2, space="PSUM"))
        v_ps = p1_ctx.enter_context(tc.tile_pool(name="vps", bufs=4, space="PSUM"))

        # warm-up matmuls: un-throttle the PE HAM clock gate while DMAs land.
        # Operands are memset junk with no DMA dependencies, so the PE starts
        # the moment its sequencer reaches main.
        warm = k_ps.tile([P, CH], f32, tag="pk", name="warm")
        for i in range(8):
            nc.tensor.matmul(warm, lhsT=wjunk[:, 0:P], rhs=wjunk,
                             start=(i == 0), stop=(i == 7))

        for c in range(NCH):
            csl = slice(c * CH, (c + 1) * CH)
            pk = [k_ps.tile([P, CH], f32, tag="pk", name=f"pk{c}_{h}")
                  for h in range(HPC)]
            pv = [v_ps.tile([P, HDC], f32, tag="pv", name=f"pv{c}_{st}")
                  for st in range(4)]
            pq = ([q_ps.tile([P, CH], f32, tag="pq", name=f"pq{c}_{h}")
                   for h in range(HPC)] if c == 0 else None)
            for d in range(DT):
                first, last = (d == 0), (d == DT - 1)
                for h in range(HPC):
                    nc.tensor.matmul(
                        pk[h], lhsT=wk_sb[:, d, h * HD:(h + 1) * HD],
                        rhs=x_sb[:, d, csl], start=first, stop=last)
                if last:
                    for h in range(HPC):
                        nc.vector.tensor_copy(kT_sb[:, h, csl], pk[h])
                if pq is not None:
                    for h in range(HPC):
                        nc.tensor.matmul(
                            pq[h], lhsT=wq_sb[:, d, h * HD:(h + 1) * HD],
                            rhs=x_sb[:, d, csl], start=first, stop=last)
                    if last:
                        for h in range(HPC):
                            nc.vector.tensor_copy(qT_sb[:, h, csl], pq[h])
                for st in range(4):
                    nc.tensor.matmul(
                        pv[st],
                        lhsT=x_sb[:, d, c * CH + st * P:c * CH + (st + 1) * P],
                        rhs=wv_sb[:, d, :], start=first, stop=last)
                if last:
                    for st in range(4):
                        # scalar engine is idle in phase 1: drain v there so
                        # chunk boundaries don't serialize on the DVE
                        nc.scalar.copy(v_sb[:, c * 4 + st, :], pv[st])

        p1_ctx.close()  # release phase-1 PSUM banks

        # ---------- Phase 2 PSUM pools ----------
        # deferred-q accum and the denominator tile are serially live within a
        # unit, so they share one rotating bank (tag "scr"); that frees a bank
        # to double-buffer the wo projection PSUM.
        wo_ps = ctx.enter_context(tc.tile_pool(name="wops", bufs=2, space="PSUM"))
        p2_ctx = ExitStack()
        sc_ps = p2_ctx.enter_context(tc.tile_pool(name="scps", bufs=2, space="PSUM"))
        o_ps = p2_ctx.enter_context(tc.tile_pool(name="ops", bufs=1, space="PSUM"))
        scr_ps = p2_ctx.enter_context(tc.tile_pool(name="scrps", bufs=1, space="PSUM"))

        def make_defq_units(c, h):
            """16 single-matmul units projecting q for (c, h); last drains."""
            csl = slice(c * CH, (c + 1) * CH)
            pqd = scr_ps.tile([P, CH], f32, tag="scr", name=f"dpq{c}_{h}")

            def unit(d):
                def emit():
                    nc.tensor.matmul(
                        pqd, lhsT=wq_sb[:, d, h * HD:(h + 1) * HD],
                        rhs=x_sb[:, d, csl],
                        start=(d == 0), stop=(d == DT - 1))
                    if d == DT - 1:
                        nc.vector.tensor_copy(qT_sb[:, h, csl], pqd)
                return emit

            return [unit(d) for d in range(DT)]

        def make_wo_units(c, ob, pool):
            """32 single-matmul units for chunk c's output projection.
            ob: [P, DT, CH] f16 SBUF chunk buffer; DMA per 4-ot quad."""
            csl = slice(c * CH, (c + 1) * CH)
            state = {}

            def unit(ot, di):
                def emit():
                    if di == 0:
                        state[ot] = pool.tile([P, CH], f32, tag="pout",
                                              name=f"po{c}_{ot}")
                    pout = state[ot]
                    nc.tensor.matmul(
                        pout, lhsT=wo_sb[:, di, ot * P:(ot + 1) * P],
                        rhs=oT_sb[:, di, csl],
                        start=(di == 0), stop=(di == HDC // P - 1))
                    if di == HDC // P - 1:
                        if ot % 2 == 0:
                            nc.vector.tensor_copy(ob[:, ot, :], pout)
                        else:
                            nc.scalar.copy(ob[:, ot, :], pout)
                        if ot % 4 == 3:
                            nc.gpsimd.dma_start(
                                out=outb[c][:, ot - 3:ot + 1, :],
                                in_=ob[:, ot - 3:ot + 1, :])
                return emit

            return [unit(ot, di) for ot in range(DT) for di in range(HDC // P)]

        # ---------- Phase 2: attention with interleaved fillers ----------
        def attention(c, h, fillers, fills_per_g):
            csl = slice(c * CH, (c + 1) * CH)
            po = o_ps.tile([P, CH], f32, tag="po", name=f"pa{c}_{h}")
            ptsum = pspool.tile([P, 2, CH], f16, tag="ptsum", name=f"pts{c}_{h}")
            fi = 0
            for g in range(KT // 2):
                psc = sc_ps.tile([P, 2, CH], f32, tag="psc", name=f"psc{c}_{h}_{g}")
                for j in range(2):
                    kj = g * 2 + j
                    nc.tensor.matmul(
                        psc[:, j, :], lhsT=kT_sb[:, h, kj * P:(kj + 1) * P],
                        rhs=qT_sb[:, h, csl], start=True, stop=True)
                # independent PE work here hides the exp latency
                for _ in range(fills_per_g[g]):
                    if fi < len(fillers):
                        fillers[fi]()
                        fi += 1
                pt = ptpool.tile([P, 2, CH], f16, tag="pt", name=f"pt{c}_{h}_{g}")
                nc.scalar.activation(
                    out=pt, in_=psc,
                    func=mybir.ActivationFunctionType.Exp, scale=SCALE)
                for j in range(2):
                    kj = g * 2 + j
                    nc.tensor.matmul(
                        po, lhsT=v_sb[:, kj, h * HD:(h + 1) * HD],
                        rhs=pt[:, j, :], start=(kj == 0), stop=(kj == KT - 1))
                if g == 0:
                    nc.vector.tensor_copy(ptsum, pt)
                else:
                    nc.vector.tensor_add(ptsum, ptsum, pt)
            while fi < len(fillers):
                fillers[fi]()
                fi += 1
            with tc.high_priority():
                ptf = pspool.tile([P, CH], f16, tag="ptf", name=f"ptf{c}_{h}")
                nc.vector.tensor_add(ptf, ptsum[:, 0, :], ptsum[:, 1, :])
                pden = scr_ps.tile([P, CH], f32, tag="scr", name=f"pd{c}_{h}")
                nc.tensor.matmul(pden, lhsT=ones, rhs=ptf, start=True, stop=True)
                rs = rspool.tile([P, CH], f32, tag="rs", name=f"rs{c}_{h}")
                nc.vector.reciprocal_approx_fast(out=rs, in_=pden)
                nc.vector.tensor_mul(oT_sb[:, h, csl], po, rs)

        obs = {}
        for c in range(NCH):
            if c > 0:
                obs[c - 1] = obpool.tile([P, DT, CH], f16, tag="ob",
                                         name=f"ob{c - 1}")
            for h in range(HPC):
                fills = []
                if c + 1 < NCH:
                    fills += make_defq_units(c + 1, h)        # 16 units
                if c > 0:
                    wo_units = make_wo_units(c - 1, obs[c - 1], wo_ps) \
                        if h == 0 else obs[(c - 1, "units")]
                    obs[(c - 1, "units")] = wo_units
                    half = len(wo_units) // 2                  # 16 units
                    fills += wo_units[:half] if h == 0 else wo_units[half:]
                n = len(fills)
                if (c, h) == (NCH - 1, HPC - 1):
                    # last unit: hold back fillers to cover the final
                    # denominator chain's latency before the tail starts
                    n = max(0, n - 3)
                base, extra = n // 8, n % 8
                pat = [base + (1 if g < extra else 0) for g in range(8)]
                attention(c, h, fills, pat)
        # tail: last chunk's output projection with deep PSUM buffering
        # (attention pools released first so four pout banks can rotate)
        p2_ctx.close()
        tail_ps = ctx.enter_context(tc.tile_pool(name="tailps", bufs=6, space="PSUM"))
        obs[NCH - 1] = obpool.tile([P, DT, CH], f16, tag="ob", name=f"ob{NCH - 1}")
        for u in make_wo_units(NCH - 1, obs[NCH - 1], tail_ps):
            u()

    nc.compile()
    return nc


def _get_nc():
    if "nc" not in _CACHE:
        _CACHE["nc"] = _build_nc()
    return _CACHE["nc"]


def make_in_maps(x, w_q, w_k, w_v, w_o):
    x = np.asarray(x, dtype=np.float32).reshape(S, D)
    w_q = np.asarray(w_q, dtype=np.float32)
    w_k = np.asarray(w_k, dtype=np.float32)
    w_v = np.asarray(w_v, dtype=np.float32)
    w_o = np.asarray(w_o, dtype=np.float32)
    xT = x.T.astype(np.float16)                      # [D, S]
    # xin[half, dt, p, j] = xT[dt*128 + p, half*1024 + j]
    xin = np.ascontiguousarray(
        xT.reshape(DT, P, 2, S // 2).transpose(2, 0, 1, 3))

    def pack_w(w_slice):                             # w_slice: [HDC, D]
        wT = w_slice.T.astype(np.float16)            # [D, HDC]
        # win[p, dt, h] = wT[dt*128 + p, h]
        return np.ascontiguousarray(wT.reshape(DT, P, HDC).transpose(1, 0, 2))

    in_maps = []
    for r in range(NCORES):
        hs = slice(r * HDC, (r + 1) * HDC)
        woT = w_o[:, hs].T.astype(np.float16)        # [HDC, D]
        # woin[p, di, o] = woT[di*128 + p, o]
        woin = np.ascontiguousarray(
            woT.reshape(HDC // P, P, D).transpose(1, 0, 2))
        in_maps.append({
            "xin": xin,
            "wqin": pack_w(w_q[hs, :]),
            "wkin": pack_w(w_k[hs, :]),
            "wvin": pack_w(w_v[hs, :]),
            "woin": woin,
        })
    return in_maps


def kernel(x, w_q, w_k, w_v, w_o):
    global LAST_RESULT
    in_maps = make_in_maps(x, w_q, w_k, w_v, w_o)
    nc = _get_nc()
    res = run_bass_kernel_spmd(nc, in_maps, core_ids=list(range(NCORES)))
    LAST_RESULT = res
    acc = np.zeros((D, S), dtype=np.float32)
    for r in res.results:
        ob = r["outb"].astype(np.float32)            # [NCH, P, DT, CH]
        # outT[ot*128 + p, c*512 + j] = ob[c, p, ot, j]
        acc += ob.transpose(2, 1, 0, 3).reshape(D, S)
    return np.ascontiguousarray(acc.T).astype(np.float32).reshape(1, S, D)
